# revision 63
# baseline (speedup 1.0000x reference)
"""Trainium2 Bass kernel for Llama-like attention (16 heads, tanh softcap, RoPE).

Sharding: tensor-parallel over heads, fully COLLECTIVE-FREE. Each of the 8
cores computes 2 heads end to end; the only cross-core combine (the sum over
heads after wo) happens on the host, so no core ever waits on another and the
device time is pure single-core compute.

Per core:
  - all inputs are host-packed to [128, ...] k-chunk-major layouts so each
    tensor is ONE contiguous full-rate DMA, issued in first-use order.
  - q/k/v projections in natural [s, d] layout, f32 PSUM. wq/wk columns are
    host-interleaved as [qh0|qh1|kh0|kh1 evens, same odds], so RoPE is 6
    contiguous [128, 256] DVE ops per s-chunk reading PSUM directly (the
    head-dim permutation to [evens|odds] cancels inside q.k).
  - attention with scores transposed ([kj, qi]) so softmaxed probabilities
    feed the PV matmul as the moving operand. tanh softcap bounds scores,
    so softmax needs no row-max pass: p = exp(50*tanh(qk/(50*sqrt(hd)))),
    l = ones-row matmul, o = p@v / l. Fully-masked diagonal spans skip the
    activations; a [zeros|triangle] mask fuses zeroing + causal masking
    into one multiply.
  - output projection is a PARTIAL over the full output width:
    outT_c = wo[local head rows, :]^T @ oT_local ([DM, S] fp16); the host
    sums the 8 partials in f32.
"""

import os
import sys

for _p in ("/root/.axon_site/_ro/trn_rl_repo", "/opt/trn_rl_repo"):
    if os.path.isdir(_p) and _p not in sys.path:
        sys.path.append(_p)

import numpy as np
import ml_dtypes
from contextlib import ExitStack

import concourse.bass as bass
import concourse.bacc as bacc
import concourse.mybir as mybir
import concourse.tile as tile
from concourse.bass_utils import run_bass_kernel_spmd
from concourse.masks import make_identity

BF16 = mybir.dt.bfloat16
F16 = mybir.dt.float16
F32 = mybir.dt.float32
NPBF16 = ml_dtypes.bfloat16

N_CORES = 8
S = 2048          # sequence length
DM = 2048         # model dim
H = 16            # heads
HD = 128          # head dim
HPC = H // N_CORES  # heads per core = 2
CW = HPC * HD     # per-core projection width = 256
P = 128
QT = 512          # query tile (free dim of attention matmuls)
NQT = S // QT     # 4 query tiles per head
NSC = S // P      # 16 sequence chunks
NKC = DM // P     # 16 contraction chunks
SOFTCAP = 50.0
C1 = 1.0 / (SOFTCAP * np.sqrt(HD))

Tanh = mybir.ActivationFunctionType.Tanh
Exp = mybir.ActivationFunctionType.Exp


def build_nc(reps=1, single=False):
    nc = bacc.Bacc("TRN2", target_bir_lowering=False, num_devices=N_CORES)

    # all inputs host-packed to [P, ...] so each is one contiguous DMA
    xq0_d = nc.dram_tensor("x_q0", [P, NKC * 512], BF16, kind="ExternalInput")
    xq1_d = nc.dram_tensor("x_q1", [P, NKC * 512], BF16, kind="ExternalInput")
    xh2_d = nc.dram_tensor("x_h2", [P, NKC * 1024], BF16, kind="ExternalInput")
    wqk_d = nc.dram_tensor("w_qk", [P, NKC * 2 * CW], BF16, kind="ExternalInput")
    wv_d = nc.dram_tensor("w_v", [P, NKC * CW], BF16, kind="ExternalInput")
    wo_d = nc.dram_tensor("wo_c", [P, HPC * DM], BF16, kind="ExternalInput")
    # cos/sin pre-packed on host to [P, NSC*4*HD/2] so one contiguous DMA each
    cos_d = nc.dram_tensor("cos_b", [P, NSC * 4 * (HD // 2)], BF16,
                           kind="ExternalInput")
    sin_d = nc.dram_tensor("sin_b", [P, NSC * 4 * (HD // 2)], BF16,
                           kind="ExternalInput")
    mask_d = nc.dram_tensor("mask", [P, 4 * P], BF16, kind="ExternalInput")
    out_d = nc.dram_tensor("outT", [DM, S], F16, kind="ExternalOutput")

    with tile.TileContext(nc) as tc:
        for _rep in range(reps):
            _emit_body(nc, tc, xq0_d, xq1_d, xh2_d, wqk_d, wv_d, wo_d,
                       cos_d, sin_d, mask_d, out_d)
    nc.compile()
    return nc


def _emit_body(nc, tc, xq0_d, xq1_d, xh2_d, wqk_d, wv_d, wo_d, cos_d,
               sin_d, mask_d, out_d):
        with ExitStack() as ctx:
            # ---------- persistent SBUF ----------
            persist = ctx.enter_context(tc.tile_pool(name="persist", bufs=1))
            qT = [persist.tile([P, S], BF16, name=f"qT{h}") for h in range(HPC)]
            kT = [persist.tile([P, S], BF16, name=f"kT{h}") for h in range(HPC)]
            v_sb = [persist.tile([P, S], BF16, name=f"v{h}") for h in range(HPC)]
            oT = [persist.tile([P, S], BF16, name=f"oT{h}") for h in range(HPC)]
            mask_sb = persist.tile([P, 4 * P], BF16, name="mask")
            ident = persist.tile([P, P], BF16, name="ident")
            ones_bf = persist.tile([P, 1], BF16, name="ones")
            # cos/sin duplicated 4x on host: one [P, 256] slice ropes
            # q-h0|q-h1|k-h0|k-h1 in a single op
            cos_sb = persist.tile([P, NSC, 4 * (HD // 2)], BF16, name="cos")
            sin_sb = persist.tile([P, NSC, 4 * (HD // 2)], BF16, name="sin")
            wo_sb = persist.tile([P, HPC, DM], BF16, name="wo")

            make_identity(nc, ident[:])
            nc.vector.memset(ones_bf[:], 1.0)

            # shared attention SBUF pools (used both interleaved with phase A
            # and in the tail); pT bufs zeroed once up front: diagonal
            # mask-muls zero stale spans, safe only for finite stale data
            thp = ctx.enter_context(tc.tile_pool(name="tanh", bufs=4))
            pp = ctx.enter_context(tc.tile_pool(name="pT", bufs=9))
            np_ = ctx.enter_context(tc.tile_pool(name="norm", bufs=2))
            # ---------- phase A: qkv projections + rope + transpose ----------
            with ExitStack() as ctxA:
                xp = ctxA.enter_context(tc.tile_pool(name="xT", bufs=1))
                wp = ctxA.enter_context(tc.tile_pool(name="w", bufs=1))
                rp = ctxA.enter_context(tc.tile_pool(name="rope", bufs=3))
                tmp = ctxA.enter_context(tc.tile_pool(name="ropetmp", bufs=4))
                qk_ps = ctxA.enter_context(
                    tc.tile_pool(name="qk_ps", bufs=1, space="PSUM")
                )
                v_ps = ctxA.enter_context(
                    tc.tile_pool(name="v_ps", bufs=1, space="PSUM")
                )
                tp_ps = ctxA.enter_context(
                    tc.tile_pool(name="tp_ps", bufs=2, space="PSUM")
                )

                # every input is one contiguous full-rate DMA, ordered by
                # first use: w_qk+x_q0 gate s-chunk 0, then w_v, cos/sin
                # (first rope), the rest of x, mask/wo (phase B/C)
                xq0_sb = xp.tile([P, NKC, 512], BF16, name="xq0")
                xq1_sb = xp.tile([P, NKC, 512], BF16, name="xq1")
                xh2_sb = xp.tile([P, NKC, 1024], BF16, name="xh2")
                wt_sb = wp.tile([P, NKC, 2 * CW], BF16, name="wqk")
                wv_sb2 = wp.tile([P, NKC, CW], BF16, name="wv")
                QK = NKC // 4
                for g in range(4):
                    k0, k1 = g * QK, (g + 1) * QK
                    nc.sync.dma_start(out=wt_sb[:, k0:k1, :],
                                      in_=wqk_d[:, k0 * 2 * CW:k1 * 2 * CW])
                    nc.sync.dma_start(out=xq0_sb[:, k0:k1, :],
                                      in_=xq0_d[:, k0 * 512:k1 * 512])
                nc.sync.dma_start(out=wv_sb2[:], in_=wv_d[:])
                nc.sync.dma_start(out=cos_sb[:], in_=cos_d[:])
                nc.sync.dma_start(out=sin_sb[:], in_=sin_d[:])
                nc.sync.dma_start(out=xq1_sb[:], in_=xq1_d[:])
                nc.sync.dma_start(out=xh2_sb[:], in_=xh2_d[:])
                nc.sync.dma_start(out=mask_sb[:], in_=mask_d[:])
                nc.sync.dma_start(out=wo_sb[:], in_=wo_d[:])

                HW = HD // 2  # 64

                def emit_proj_chunk(sc):
                    ps = qk_ps.tile([P, 2 * CW], F32, name="qk")
                    psv = v_ps.tile([P, CW], F32, name="v")
                    xsb = xq0_sb if sc < 4 else (xq1_sb if sc < 8 else xh2_sb)
                    xo = (sc % 4) * P if sc < 8 else (sc - 8) * P
                    for k in range(NKC):
                        lhsT = xsb[:, k, xo:xo + P]
                        nc.tensor.matmul(
                            ps[:, 0:512], lhsT, wt_sb[:, k, :],
                            start=(k == 0), stop=(k == NKC - 1),
                        )
                    for k in range(NKC):
                        lhsT = xsb[:, k, xo:xo + P]
                        nc.tensor.matmul(
                            psv[:], lhsT, wv_sb2[:, k, :],
                            start=(k == 0), stop=(k == NKC - 1),
                        )
                    for h in range(HPC):
                        nc.scalar.copy(
                            v_sb[h][:, sc * P:(sc + 1) * P],
                            psv[:, h * HD:(h + 1) * HD],
                        )
                    c_ap = cos_sb[:, sc, :]
                    s_ap = sin_sb[:, sc, :]
                    # rope reads the projection PSUM directly; the host packs
                    # w_qk columns as [qh0|qh1|kh0|kh1 evens, same odds] so
                    # x0/x1 are contiguous [P, 256] and one mul covers q+k of
                    # both heads
                    qkst = rp.tile([P, 512], BF16, name="qkst")
                    nc.vector.tensor_copy(qkst[:], ps[:, 0:512])
                    x0 = qkst[:, 0:256]
                    x1 = qkst[:, 256:512]
                    rot = rp.tile([P, 4, 2, HW], BF16, name="rot")
                    t1 = tmp.tile([P, 256], BF16, name="t1")
                    t2 = tmp.tile([P, 256], BF16, name="t2")
                    nc.vector.tensor_mul(t1[:], x0, c_ap)
                    nc.vector.tensor_mul(t2[:], x1, s_ap)
                    nc.vector.tensor_sub(rot[:, :, 0, :], t1[:], t2[:])
                    t3 = tmp.tile([P, 256], BF16, name="t3")
                    t4 = tmp.tile([P, 256], BF16, name="t4")
                    nc.vector.tensor_mul(t3[:], x0, s_ap)
                    nc.vector.tensor_mul(t4[:], x1, c_ap)
                    nc.vector.tensor_add(rot[:, :, 1, :], t3[:], t4[:])
                    # head-dim order becomes [evens, odds] for both q and k,
                    # which cancels in q.k
                    for srcg, rotT in ((0, qT), (1, kT)):
                        for h in range(HPC):
                            g = srcg * 2 + h
                            tp = tp_ps.tile([P, P], BF16, name="tp")
                            nc.tensor.transpose(
                                tp[:], rot[:, g, :, :], ident[:])
                            nc.scalar.copy(
                                rotT[h][:, sc * P:(sc + 1) * P], tp[:])

                def softcap(pT, sp, lo, hi, th_pool):
                    """pT[:, lo:hi] = exp(50*tanh(sp*C1)), both on ACT."""
                    th = th_pool.tile([P, 2 * QT], F32, name="th")
                    nc.scalar.activation(th[:, lo:hi], sp[:, lo:hi], Tanh,
                                         scale=C1)
                    nc.scalar.activation(pT[:, lo:hi], th[:, lo:hi], Exp,
                                         scale=SOFTCAP)

                def sc_block(h, t, s_pool, th_pool, p_pool):
                    """scores + softcapped exp for every causal chunk-pair of
                    query tile t; returns the probability tiles for pv_block"""
                    q_ap = qT[h][:, t * QT:(t + 1) * QT]
                    plist = []
                    for p in range(2 * t + 2):
                        sp = s_pool.tile([P, 2 * QT], F32, name="sp")
                        for i in range(2):
                            kc = 2 * p + i
                            nc.tensor.matmul(
                                sp[:, i * QT:(i + 1) * QT],
                                kT[h][:, kc * P:(kc + 1) * P], q_ap,
                                start=True, stop=True,
                            )
                        pT = p_pool.tile([P, 2 * QT], BF16, name="pTt")
                        u0 = 2 * (p - 2 * t)
                        if u0 < 0:
                            softcap(pT, sp, 0, 2 * QT, th_pool)
                        else:
                            # diagonal pair: chunk u=u0+i is fully masked for
                            # in-tile queries < 128*u; softcap only the live
                            # span, then one mul against [zeros|triangle]
                            # zeroes the masked span and applies the triangle
                            for i in range(2):
                                zs = (u0 + i) * P
                                c0 = i * QT
                                softcap(pT, sp, c0 + zs, c0 + QT, th_pool)
                                nc.gpsimd.tensor_mul(
                                    pT[:, c0:c0 + zs + P],
                                    pT[:, c0:c0 + zs + P],
                                    mask_sb[:, 3 * P - zs:4 * P])
                        plist.append(pT)
                    return plist

                def pv_block(h, t, plist, o_pool, l_pool, n_pool):
                    """o = p@v accumulation, l = ones-row sums, then the
                    softmax normalization into oT"""
                    o_acc = o_pool.tile([P, QT], F32, name="o_acc")
                    l_acc = l_pool.tile([1, QT], F32, name="l_acc")
                    npair = len(plist)
                    for p, pT in enumerate(plist):
                        for i in range(2):
                            kc = 2 * p + i
                            last = (p == npair - 1 and i == 1)
                            nc.tensor.matmul(
                                o_acc[:],
                                v_sb[h][:, kc * P:(kc + 1) * P],
                                pT[:, i * QT:(i + 1) * QT],
                                start=(kc == 0), stop=last,
                            )
                            nc.tensor.matmul(
                                l_acc[:], ones_bf[:, 0:1],
                                pT[:, i * QT:(i + 1) * QT],
                                start=(kc == 0), stop=last,
                            )
                    recip = n_pool.tile([1, QT], F32, name="recip")
                    nc.vector.reciprocal(recip[:], l_acc[:])
                    bcast = n_pool.tile([P, QT], F32, name="bcast")
                    nc.gpsimd.partition_broadcast(bcast[:], recip[:])
                    nc.vector.tensor_mul(
                        oT[h][:, t * QT:(t + 1) * QT], o_acc[:], bcast[:])

                for sc in range(4):
                    emit_proj_chunk(sc)

                # query tiles t0..t2 interleave with the remaining projection
                # chunks: tile t needs only s-chunks 0..4t+3, and ACT chews
                # tanh/exp while PE is busy projecting
                with ExitStack() as ctxAB:
                    s1 = ctxAB.enter_context(
                        tc.tile_pool(name="s1_ps", bufs=1, space="PSUM"))
                    o1 = ctxAB.enter_context(
                        tc.tile_pool(name="o1_ps", bufs=1, space="PSUM"))
                    l1 = ctxAB.enter_context(
                        tc.tile_pool(name="l1_ps", bufs=1, space="PSUM"))
                    th1, pp1, np1 = thp, pp, np_

                    emit_proj_chunk(4)
                    pl = sc_block(0, 0, s1, th1, pp1)
                    emit_proj_chunk(5)
                    pv_block(0, 0, pl, o1, l1, np1)
                    pl = sc_block(1, 0, s1, th1, pp1)
                    emit_proj_chunk(6)
                    pv_block(1, 0, pl, o1, l1, np1)
                    emit_proj_chunk(7)
                    emit_proj_chunk(8)
                    pl = sc_block(0, 1, s1, th1, pp1)
                    emit_proj_chunk(9)
                    pv_block(0, 1, pl, o1, l1, np1)
                    pl = sc_block(1, 1, s1, th1, pp1)
                    emit_proj_chunk(10)
                    pv_block(1, 1, pl, o1, l1, np1)
                    emit_proj_chunk(11)
                    emit_proj_chunk(12)
                    pl = sc_block(0, 2, s1, th1, pp1)
                    emit_proj_chunk(13)
                    pv_block(0, 2, pl, o1, l1, np1)
                    pl = sc_block(1, 2, s1, th1, pp1)
                    emit_proj_chunk(14)
                    pv_block(1, 2, pl, o1, l1, np1)
                    emit_proj_chunk(15)

            # ---------- phase B tail: the last query tile per head ----------
            with ExitStack() as ctxB:
                s_ps = ctxB.enter_context(
                    tc.tile_pool(name="s_ps", bufs=3, space="PSUM"))
                o_ps = ctxB.enter_context(
                    tc.tile_pool(name="o_ps", bufs=1, space="PSUM"))
                l_ps = ctxB.enter_context(
                    tc.tile_pool(name="l_ps", bufs=1, space="PSUM"))
                for h in range(HPC):
                    pl = sc_block(h, 3, s_ps, thp, pp)
                    pv_block(h, 3, pl, o_ps, l_ps, np_)

            # ---------- phase C: partial output projection (no collective) --
            with ExitStack() as ctxC:
                outp = ctxC.enter_context(tc.tile_pool(name="out", bufs=4))
                wo_ps = ctxC.enter_context(
                    tc.tile_pool(name="wo_ps", bufs=6, space="PSUM"))

                # matmul PSUM outputs are capped at 512 f32 per partition;
                # stage four 512-wide accs into one [P, S] row, one DMA per m
                for m in range(NKC):
                    osb = outp.tile([P, S], F16, name="osb")
                    for n in range(NQT):
                        acc = wo_ps.tile([P, QT], F32, name="acc")
                        for j in range(HPC):
                            nc.tensor.matmul(
                                acc[:], wo_sb[:, j, m * P:(m + 1) * P],
                                oT[j][:, n * QT:(n + 1) * QT],
                                start=(j == 0), stop=(j == HPC - 1),
                            )
                        if n % 2 == 0:
                            nc.scalar.copy(osb[:, n * QT:(n + 1) * QT], acc[:])
                        else:
                            nc.vector.tensor_copy(
                                osb[:, n * QT:(n + 1) * QT], acc[:])
                    nc.sync.dma_start(out=out_d[m * P:(m + 1) * P, :],
                                      in_=osb[:])


_NC_CACHE = None


def _get_nc():
    global _NC_CACHE
    if _NC_CACHE is None:
        _NC_CACHE = build_nc()
    return _NC_CACHE


def _kpack(a, kc, kw):
    """[kc*128, kc_width] row-major -> [128, kc*kc_width] k-chunk-packed"""
    a = np.ascontiguousarray(a)
    return np.ascontiguousarray(
        a.reshape(kc, P, kw).transpose(1, 0, 2).reshape(P, kc * kw))


def make_in_maps(x, wq, wk, wv, wo, freqs_cos, freqs_sin):
    x = np.asarray(x, np.float32).reshape(S, DM)
    wq = np.asarray(wq, np.float32)
    wk = np.asarray(wk, np.float32)
    wv = np.asarray(wv, np.float32)
    wo = np.asarray(wo, np.float32)
    xT = np.ascontiguousarray(x.T).astype(NPBF16)  # [DM, S]
    x_q0 = _kpack(xT[:, 0:512], NKC, 512)
    x_q1 = _kpack(xT[:, 512:1024], NKC, 512)
    x_h2 = _kpack(xT[:, 1024:S], NKC, 1024)

    # cos/sin packed to [P, NSC*4*64]: dest[p, (sc*4+g)*64+f] = src[sc*128+p, f]
    # (duplicated over g=0..3 so one [P,256] slice ropes qh0|qh1|kh0|kh1)
    def _pack(t):
        t = np.asarray(t, np.float32).reshape(NSC, P, HD // 2)
        t4 = np.repeat(t[:, :, None, :], 4, axis=2)  # [NSC, P, 4, 64]
        return np.ascontiguousarray(
            t4.transpose(1, 0, 2, 3).reshape(P, NSC * 4 * (HD // 2))
        ).astype(NPBF16)

    cos_b = _pack(freqs_cos)
    sin_b = _pack(freqs_sin)
    # [P, 4P] = [zeros(3P) | triangle(P)]: slicing the tail [3P-zs : 4P]
    # gives zs zeros then the causal triangle (keep key-off i <= query-off j)
    i_idx = np.arange(P)[:, None]
    j_idx = np.arange(4 * P)[None, :]
    mask = (np.logical_and(j_idx >= 3 * P, i_idx <= j_idx - 3 * P)
            ).astype(NPBF16)
    ev = np.arange(0, HD, 2)
    od = np.arange(1, HD, 2)
    in_maps = []
    for c in range(N_CORES):
        g0, g1 = HPC * c, HPC * c + 1
        # w_qk columns: [qh0 ev | qh1 ev | kh0 ev | kh1 ev |
        #                qh0 od | qh1 od | kh0 od | kh1 od], 64 each
        w_qk = np.concatenate([
            wq[:, g0 * HD + ev], wq[:, g1 * HD + ev],
            wk[:, g0 * HD + ev], wk[:, g1 * HD + ev],
            wq[:, g0 * HD + od], wq[:, g1 * HD + od],
            wk[:, g0 * HD + od], wk[:, g1 * HD + od],
        ], axis=1).astype(NPBF16)
        cs = slice(c * CW, (c + 1) * CW)
        w_v = wv[:, cs].astype(NPBF16)
        # rows of wo for this core's heads (2c, 2c+1); v/o are not roped,
        # so natural head-dim order
        wo_c = wo[cs, :].astype(NPBF16)
        in_maps.append({
            "x_q0": x_q0,
            "x_q1": x_q1,
            "x_h2": x_h2,
            "w_qk": _kpack(w_qk, NKC, 2 * CW),
            "w_v": _kpack(w_v, NKC, CW),
            "wo_c": _kpack(wo_c, HPC, DM),
            "cos_b": cos_b,
            "sin_b": sin_b,
            "mask": mask,
        })
    return in_maps


def assemble_output(results):
    acc = np.zeros((DM, S), np.float32)
    for r in results:
        acc += np.asarray(r["outT"], np.float32)
    return np.ascontiguousarray(acc.T).reshape(1, S, DM)


def kernel(x, wq, wk, wv, wo, freqs_cos, freqs_sin):
    nc = _get_nc()
    in_maps = make_in_maps(x, wq, wk, wv, wo, freqs_cos, freqs_sin)
    res = run_bass_kernel_spmd(nc, in_maps, core_ids=list(range(N_CORES)))
    return assemble_output(res.results)


if __name__ == "__main__":
    rng = np.random.default_rng(0)
    ins = {
        "x": rng.standard_normal((1, S, DM)).astype(np.float32),
        "wq": (rng.standard_normal((DM, DM)) / np.sqrt(DM)).astype(np.float32),
        "wk": (rng.standard_normal((DM, DM)) / np.sqrt(DM)).astype(np.float32),
        "wv": (rng.standard_normal((DM, DM)) / np.sqrt(DM)).astype(np.float32),
        "wo": (rng.standard_normal((DM, DM)) / np.sqrt(DM)).astype(np.float32),
        "freqs_cos": rng.random((S, HD // 2)).astype(np.float32),
        "freqs_sin": rng.random((S, HD // 2)).astype(np.float32),
    }
    out = kernel(**ins)
    print("out", out.shape, out.dtype, np.abs(out).mean())


# revision 66
# speedup vs baseline: 1.0219x; 1.0219x over previous
"""Trainium2 Bass kernel for Llama-like attention (16 heads, tanh softcap, RoPE).

Sharding: tensor-parallel over heads, fully COLLECTIVE-FREE. Each of the 8
cores computes 2 heads end to end; the only cross-core combine (the sum over
heads after wo) happens on the host, so no core ever waits on another and the
device time is pure single-core compute.

Per core:
  - all inputs are host-packed to [128, ...] k-chunk-major layouts so each
    tensor is ONE contiguous full-rate DMA, issued in first-use order.
  - q/k/v projections in natural [s, d] layout, f32 PSUM. wq/wk columns are
    host-interleaved as [qh0|qh1|kh0|kh1 evens, same odds], so RoPE is 6
    contiguous [128, 256] DVE ops per s-chunk reading PSUM directly (the
    head-dim permutation to [evens|odds] cancels inside q.k).
  - attention with scores transposed ([kj, qi]) so softmaxed probabilities
    feed the PV matmul as the moving operand. tanh softcap bounds scores,
    so softmax needs no row-max pass: p = exp(50*tanh(qk/(50*sqrt(hd)))),
    l = ones-row matmul, o = p@v / l. Fully-masked diagonal spans skip the
    activations; a [zeros|triangle] mask fuses zeroing + causal masking
    into one multiply.
  - output projection is a PARTIAL over the full output width:
    outT_c = wo[local head rows, :]^T @ oT_local ([DM, S] fp16); the host
    sums the 8 partials in f32.
"""

import os
import sys

for _p in ("/root/.axon_site/_ro/trn_rl_repo", "/opt/trn_rl_repo"):
    if os.path.isdir(_p) and _p not in sys.path:
        sys.path.append(_p)

import numpy as np
import ml_dtypes
from contextlib import ExitStack

import concourse.bass as bass
import concourse.bacc as bacc
import concourse.mybir as mybir
import concourse.tile as tile
from concourse.bass_utils import run_bass_kernel_spmd
from concourse.masks import make_identity

BF16 = mybir.dt.bfloat16
F16 = mybir.dt.float16
F32 = mybir.dt.float32
NPBF16 = ml_dtypes.bfloat16

N_CORES = 8
S = 2048          # sequence length
DM = 2048         # model dim
H = 16            # heads
HD = 128          # head dim
HPC = H // N_CORES  # heads per core = 2
CW = HPC * HD     # per-core projection width = 256
P = 128
QT = 512          # query tile (free dim of attention matmuls)
NQT = S // QT     # 4 query tiles per head
NSC = S // P      # 16 sequence chunks
NKC = DM // P     # 16 contraction chunks
SOFTCAP = 50.0
C1 = 1.0 / (SOFTCAP * np.sqrt(HD))

Tanh = mybir.ActivationFunctionType.Tanh
Exp = mybir.ActivationFunctionType.Exp


def build_nc(reps=1, single=False):
    nc = bacc.Bacc("TRN2", target_bir_lowering=False, num_devices=N_CORES)

    # all inputs host-packed to [P, ...] so each is one contiguous DMA
    xq0_d = nc.dram_tensor("x_q0", [P, NKC * 512], BF16, kind="ExternalInput")
    xq1_d = nc.dram_tensor("x_q1", [P, NKC * 512], BF16, kind="ExternalInput")
    xh2_d = nc.dram_tensor("x_h2", [P, NKC * 1024], BF16, kind="ExternalInput")
    wqk_d = nc.dram_tensor("w_qk", [P, NKC * 2 * CW], BF16, kind="ExternalInput")
    wv_d = nc.dram_tensor("w_v", [P, NKC * CW], BF16, kind="ExternalInput")
    wo_d = nc.dram_tensor("wo_c", [P, HPC * DM], BF16, kind="ExternalInput")
    # cos/sin pre-packed on host to [P, NSC*4*HD/2] so one contiguous DMA each
    cos_d = nc.dram_tensor("cos_b", [P, NSC * 4 * (HD // 2)], BF16,
                           kind="ExternalInput")
    sin_d = nc.dram_tensor("sin_b", [P, NSC * 4 * (HD // 2)], BF16,
                           kind="ExternalInput")
    mask_d = nc.dram_tensor("mask", [P, 4 * P], BF16, kind="ExternalInput")
    out_d = nc.dram_tensor("outT", [DM, S], F16, kind="ExternalOutput")

    with tile.TileContext(nc) as tc:
        for _rep in range(reps):
            _emit_body(nc, tc, xq0_d, xq1_d, xh2_d, wqk_d, wv_d, wo_d,
                       cos_d, sin_d, mask_d, out_d)
    nc.compile()
    return nc


def _emit_body(nc, tc, xq0_d, xq1_d, xh2_d, wqk_d, wv_d, wo_d, cos_d,
               sin_d, mask_d, out_d):
        with ExitStack() as ctx:
            # ---------- persistent SBUF ----------
            persist = ctx.enter_context(tc.tile_pool(name="persist", bufs=1))
            qT = [persist.tile([P, S], BF16, name=f"qT{h}") for h in range(HPC)]
            kT = [persist.tile([P, S], BF16, name=f"kT{h}") for h in range(HPC)]
            v_sb = [persist.tile([P, S], BF16, name=f"v{h}") for h in range(HPC)]
            oT = [persist.tile([P, S], BF16, name=f"oT{h}") for h in range(HPC)]
            mask_sb = persist.tile([P, 4 * P], BF16, name="mask")
            ident = persist.tile([P, P], BF16, name="ident")
            ones_bf = persist.tile([P, 1], BF16, name="ones")
            # cos/sin duplicated 4x on host: one [P, 256] slice ropes
            # q-h0|q-h1|k-h0|k-h1 in a single op
            cos_sb = persist.tile([P, NSC, 4 * (HD // 2)], BF16, name="cos")
            sin_sb = persist.tile([P, NSC, 4 * (HD // 2)], BF16, name="sin")
            wo_sb = persist.tile([P, HPC, DM], BF16, name="wo")

            make_identity(nc, ident[:])
            nc.vector.memset(ones_bf[:], 1.0)

            # shared attention SBUF pools (used both interleaved with phase A
            # and in the tail); pT bufs zeroed once up front: diagonal
            # mask-muls zero stale spans, safe only for finite stale data
            thp = ctx.enter_context(tc.tile_pool(name="tanh", bufs=4))
            pp = ctx.enter_context(tc.tile_pool(name="pT", bufs=9))
            np_ = ctx.enter_context(tc.tile_pool(name="norm", bufs=2))
            outp = ctx.enter_context(tc.tile_pool(name="out", bufs=3))

            def emit_c_half(m, nbase, acc_pool, act_copy, st_pool=None):
                """one [P, 1024] block of the partial output projection: two
                512-wide PSUM accs staged into one fp16 tile, one DMA. Copies
                go to DVE when ACT is the busy engine (attention tail)."""
                osb = (st_pool or outp).tile([P, 2 * QT], F16, name="osb")
                for i in range(2):
                    n = nbase + i
                    acc = acc_pool.tile([P, QT], F32, name="acc")
                    for j in range(HPC):
                        nc.tensor.matmul(
                            acc[:], wo_sb[:, j, m * P:(m + 1) * P],
                            oT[j][:, n * QT:(n + 1) * QT],
                            start=(j == 0), stop=(j == HPC - 1),
                        )
                    if act_copy and i == 0:
                        nc.scalar.copy(osb[:, 0:QT], acc[:])
                    else:
                        nc.vector.tensor_copy(
                            osb[:, i * QT:(i + 1) * QT], acc[:])
                nc.sync.dma_start(
                    out=out_d[m * P:(m + 1) * P,
                              nbase * QT:(nbase + 2) * QT],
                    in_=osb[:])
            # ---------- phase A: qkv projections + rope + transpose ----------
            with ExitStack() as ctxA:
                xp = ctxA.enter_context(tc.tile_pool(name="xT", bufs=1))
                wp = ctxA.enter_context(tc.tile_pool(name="w", bufs=1))
                rp = ctxA.enter_context(tc.tile_pool(name="rope", bufs=3))
                tmp = ctxA.enter_context(tc.tile_pool(name="ropetmp", bufs=4))
                qk_ps = ctxA.enter_context(
                    tc.tile_pool(name="qk_ps", bufs=1, space="PSUM")
                )
                v_ps = ctxA.enter_context(
                    tc.tile_pool(name="v_ps", bufs=1, space="PSUM")
                )
                tp_ps = ctxA.enter_context(
                    tc.tile_pool(name="tp_ps", bufs=2, space="PSUM")
                )

                # every input is one contiguous full-rate DMA, ordered by
                # first use: w_qk+x_q0 gate s-chunk 0, then w_v, cos/sin
                # (first rope), the rest of x, mask/wo (phase B/C)
                xq0_sb = xp.tile([P, NKC, 512], BF16, name="xq0")
                xq1_sb = xp.tile([P, NKC, 512], BF16, name="xq1")
                xh2_sb = xp.tile([P, NKC, 1024], BF16, name="xh2")
                wt_sb = wp.tile([P, NKC, 2 * CW], BF16, name="wqk")
                wv_sb2 = wp.tile([P, NKC, CW], BF16, name="wv")
                QK = NKC // 4
                for g in range(4):
                    k0, k1 = g * QK, (g + 1) * QK
                    nc.sync.dma_start(out=wt_sb[:, k0:k1, :],
                                      in_=wqk_d[:, k0 * 2 * CW:k1 * 2 * CW])
                    nc.sync.dma_start(out=xq0_sb[:, k0:k1, :],
                                      in_=xq0_d[:, k0 * 512:k1 * 512])
                nc.sync.dma_start(out=wv_sb2[:], in_=wv_d[:])
                nc.sync.dma_start(out=cos_sb[:], in_=cos_d[:])
                nc.sync.dma_start(out=sin_sb[:], in_=sin_d[:])
                nc.sync.dma_start(out=xq1_sb[:], in_=xq1_d[:])
                nc.sync.dma_start(out=xh2_sb[:], in_=xh2_d[:])
                nc.sync.dma_start(out=mask_sb[:], in_=mask_d[:])
                nc.sync.dma_start(out=wo_sb[:], in_=wo_d[:])

                HW = HD // 2  # 64

                def emit_proj_chunk(sc):
                    ps = qk_ps.tile([P, 2 * CW], F32, name="qk")
                    psv = v_ps.tile([P, CW], F32, name="v")
                    xsb = xq0_sb if sc < 4 else (xq1_sb if sc < 8 else xh2_sb)
                    xo = (sc % 4) * P if sc < 8 else (sc - 8) * P
                    for k in range(NKC):
                        lhsT = xsb[:, k, xo:xo + P]
                        nc.tensor.matmul(
                            ps[:, 0:512], lhsT, wt_sb[:, k, :],
                            start=(k == 0), stop=(k == NKC - 1),
                        )
                    for k in range(NKC):
                        lhsT = xsb[:, k, xo:xo + P]
                        nc.tensor.matmul(
                            psv[:], lhsT, wv_sb2[:, k, :],
                            start=(k == 0), stop=(k == NKC - 1),
                        )
                    for h in range(HPC):
                        nc.scalar.copy(
                            v_sb[h][:, sc * P:(sc + 1) * P],
                            psv[:, h * HD:(h + 1) * HD],
                        )
                    c_ap = cos_sb[:, sc, :]
                    s_ap = sin_sb[:, sc, :]
                    # rope reads the projection PSUM directly; the host packs
                    # w_qk columns as [qh0|qh1|kh0|kh1 evens, same odds] so
                    # x0/x1 are contiguous [P, 256] and one mul covers q+k of
                    # both heads
                    qkst = rp.tile([P, 512], BF16, name="qkst")
                    nc.vector.tensor_copy(qkst[:], ps[:, 0:512])
                    x0 = qkst[:, 0:256]
                    x1 = qkst[:, 256:512]
                    rot = rp.tile([P, 4, 2, HW], BF16, name="rot")
                    t1 = tmp.tile([P, 256], BF16, name="t1")
                    t2 = tmp.tile([P, 256], BF16, name="t2")
                    nc.vector.tensor_mul(t1[:], x0, c_ap)
                    nc.vector.tensor_mul(t2[:], x1, s_ap)
                    nc.vector.tensor_sub(rot[:, :, 0, :], t1[:], t2[:])
                    t3 = tmp.tile([P, 256], BF16, name="t3")
                    t4 = tmp.tile([P, 256], BF16, name="t4")
                    nc.vector.tensor_mul(t3[:], x0, s_ap)
                    nc.vector.tensor_mul(t4[:], x1, c_ap)
                    nc.vector.tensor_add(rot[:, :, 1, :], t3[:], t4[:])
                    # head-dim order becomes [evens, odds] for both q and k,
                    # which cancels in q.k
                    for srcg, rotT in ((0, qT), (1, kT)):
                        for h in range(HPC):
                            g = srcg * 2 + h
                            tp = tp_ps.tile([P, P], BF16, name="tp")
                            nc.tensor.transpose(
                                tp[:], rot[:, g, :, :], ident[:])
                            nc.scalar.copy(
                                rotT[h][:, sc * P:(sc + 1) * P], tp[:])

                def softcap(pT, sp, lo, hi, th_pool):
                    """pT[:, lo:hi] = exp(50*tanh(sp*C1)), both on ACT."""
                    th = th_pool.tile([P, 2 * QT], F32, name="th")
                    nc.scalar.activation(th[:, lo:hi], sp[:, lo:hi], Tanh,
                                         scale=C1)
                    nc.scalar.activation(pT[:, lo:hi], th[:, lo:hi], Exp,
                                         scale=SOFTCAP)

                def sc_block(h, t, s_pool, th_pool, p_pool):
                    """scores + softcapped exp for every causal chunk-pair of
                    query tile t; returns the probability tiles for pv_block"""
                    q_ap = qT[h][:, t * QT:(t + 1) * QT]
                    plist = []
                    for p in range(2 * t + 2):
                        sp = s_pool.tile([P, 2 * QT], F32, name="sp")
                        for i in range(2):
                            kc = 2 * p + i
                            nc.tensor.matmul(
                                sp[:, i * QT:(i + 1) * QT],
                                kT[h][:, kc * P:(kc + 1) * P], q_ap,
                                start=True, stop=True,
                            )
                        pT = p_pool.tile([P, 2 * QT], BF16, name="pTt")
                        u0 = 2 * (p - 2 * t)
                        if u0 < 0:
                            softcap(pT, sp, 0, 2 * QT, th_pool)
                        else:
                            # diagonal pair: chunk u=u0+i is fully masked for
                            # in-tile queries < 128*u; softcap only the live
                            # span, then one mul against [zeros|triangle]
                            # zeroes the masked span and applies the triangle
                            for i in range(2):
                                zs = (u0 + i) * P
                                c0 = i * QT
                                softcap(pT, sp, c0 + zs, c0 + QT, th_pool)
                                nc.gpsimd.tensor_mul(
                                    pT[:, c0:c0 + zs + P],
                                    pT[:, c0:c0 + zs + P],
                                    mask_sb[:, 3 * P - zs:4 * P])
                        plist.append(pT)
                    return plist

                def pv_block(h, t, plist, o_pool, l_pool, n_pool):
                    """o = p@v accumulation, l = ones-row sums, then the
                    softmax normalization into oT"""
                    o_acc = o_pool.tile([P, QT], F32, name="o_acc")
                    l_acc = l_pool.tile([1, QT], F32, name="l_acc")
                    npair = len(plist)
                    for p, pT in enumerate(plist):
                        for i in range(2):
                            kc = 2 * p + i
                            last = (p == npair - 1 and i == 1)
                            nc.tensor.matmul(
                                o_acc[:],
                                v_sb[h][:, kc * P:(kc + 1) * P],
                                pT[:, i * QT:(i + 1) * QT],
                                start=(kc == 0), stop=last,
                            )
                            nc.tensor.matmul(
                                l_acc[:], ones_bf[:, 0:1],
                                pT[:, i * QT:(i + 1) * QT],
                                start=(kc == 0), stop=last,
                            )
                    recip = n_pool.tile([1, QT], F32, name="recip")
                    nc.vector.reciprocal(recip[:], l_acc[:])
                    bcast = n_pool.tile([P, QT], F32, name="bcast")
                    nc.gpsimd.partition_broadcast(bcast[:], recip[:])
                    nc.vector.tensor_mul(
                        oT[h][:, t * QT:(t + 1) * QT], o_acc[:], bcast[:])

                for sc in range(4):
                    emit_proj_chunk(sc)

                # query tiles t0..t2 interleave with the remaining projection
                # chunks: tile t needs only s-chunks 0..4t+3, and ACT chews
                # tanh/exp while PE is busy projecting
                with ExitStack() as ctxAB:
                    s1 = ctxAB.enter_context(
                        tc.tile_pool(name="s1_ps", bufs=1, space="PSUM"))
                    o1 = ctxAB.enter_context(
                        tc.tile_pool(name="o1_ps", bufs=1, space="PSUM"))
                    l1 = ctxAB.enter_context(
                        tc.tile_pool(name="l1_ps", bufs=1, space="PSUM"))
                    th1, pp1, np1 = thp, pp, np_

                    emit_proj_chunk(4)
                    pl = sc_block(0, 0, s1, th1, pp1)
                    emit_proj_chunk(5)
                    pv_block(0, 0, pl, o1, l1, np1)
                    pl = sc_block(1, 0, s1, th1, pp1)
                    emit_proj_chunk(6)
                    pv_block(1, 0, pl, o1, l1, np1)
                    emit_proj_chunk(7)
                    emit_proj_chunk(8)
                    pl = sc_block(0, 1, s1, th1, pp1)
                    emit_proj_chunk(9)
                    pv_block(0, 1, pl, o1, l1, np1)
                    pl = sc_block(1, 1, s1, th1, pp1)
                    emit_proj_chunk(10)
                    pv_block(1, 1, pl, o1, l1, np1)
                    emit_proj_chunk(11)
                    emit_proj_chunk(12)
                    pl = sc_block(0, 2, s1, th1, pp1)
                    emit_proj_chunk(13)
                    pv_block(0, 2, pl, o1, l1, np1)
                    pl = sc_block(1, 2, s1, th1, pp1)
                    emit_proj_chunk(14)
                    pv_block(1, 2, pl, o1, l1, np1)
                    emit_proj_chunk(15)

            # ---------- phase B tail: the last query tile per head ----------
            with ExitStack() as ctxB:
                s_ps = ctxB.enter_context(
                    tc.tile_pool(name="s_ps", bufs=2, space="PSUM"))
                o_ps = ctxB.enter_context(
                    tc.tile_pool(name="o_ps", bufs=1, space="PSUM"))
                l_ps = ctxB.enter_context(
                    tc.tile_pool(name="l_ps", bufs=1, space="PSUM"))
                woB_ps = ctxB.enter_context(
                    tc.tile_pool(name="woB_ps", bufs=2, space="PSUM"))
                # output columns for query tiles t0/t1 are final, so the
                # first output-projection half fills the tail's PE slack
                # (copies on DVE — ACT is the tail's bottleneck)
                pl = sc_block(0, 3, s_ps, thp, pp)
                for m in range(0, NKC // 2):
                    emit_c_half(m, 0, woB_ps, act_copy=False)
                pv_block(0, 3, pl, o_ps, l_ps, np_)
                pl = sc_block(1, 3, s_ps, thp, pp)
                for m in range(NKC // 2, NKC):
                    emit_c_half(m, 0, woB_ps, act_copy=False)
                pv_block(1, 3, pl, o_ps, l_ps, np_)

            # ---------- phase C: remaining output projection half ----------
            with ExitStack() as ctxC:
                wo_ps = ctxC.enter_context(
                    tc.tile_pool(name="wo_ps", bufs=6, space="PSUM"))
                outp2 = ctxC.enter_context(tc.tile_pool(name="out2", bufs=6))
                for m in range(NKC):
                    emit_c_half(m, 2, wo_ps, act_copy=True, st_pool=outp2)


_NC_CACHE = None


def _get_nc():
    global _NC_CACHE
    if _NC_CACHE is None:
        _NC_CACHE = build_nc()
    return _NC_CACHE


def _kpack(a, kc, kw):
    """[kc*128, kc_width] row-major -> [128, kc*kc_width] k-chunk-packed"""
    a = np.ascontiguousarray(a)
    return np.ascontiguousarray(
        a.reshape(kc, P, kw).transpose(1, 0, 2).reshape(P, kc * kw))


def make_in_maps(x, wq, wk, wv, wo, freqs_cos, freqs_sin):
    x = np.asarray(x, np.float32).reshape(S, DM)
    wq = np.asarray(wq, np.float32)
    wk = np.asarray(wk, np.float32)
    wv = np.asarray(wv, np.float32)
    wo = np.asarray(wo, np.float32)
    xT = np.ascontiguousarray(x.T).astype(NPBF16)  # [DM, S]
    x_q0 = _kpack(xT[:, 0:512], NKC, 512)
    x_q1 = _kpack(xT[:, 512:1024], NKC, 512)
    x_h2 = _kpack(xT[:, 1024:S], NKC, 1024)

    # cos/sin packed to [P, NSC*4*64]: dest[p, (sc*4+g)*64+f] = src[sc*128+p, f]
    # (duplicated over g=0..3 so one [P,256] slice ropes qh0|qh1|kh0|kh1)
    def _pack(t):
        t = np.asarray(t, np.float32).reshape(NSC, P, HD // 2)
        t4 = np.repeat(t[:, :, None, :], 4, axis=2)  # [NSC, P, 4, 64]
        return np.ascontiguousarray(
            t4.transpose(1, 0, 2, 3).reshape(P, NSC * 4 * (HD // 2))
        ).astype(NPBF16)

    cos_b = _pack(freqs_cos)
    sin_b = _pack(freqs_sin)
    # [P, 4P] = [zeros(3P) | triangle(P)]: slicing the tail [3P-zs : 4P]
    # gives zs zeros then the causal triangle (keep key-off i <= query-off j)
    i_idx = np.arange(P)[:, None]
    j_idx = np.arange(4 * P)[None, :]
    mask = (np.logical_and(j_idx >= 3 * P, i_idx <= j_idx - 3 * P)
            ).astype(NPBF16)
    ev = np.arange(0, HD, 2)
    od = np.arange(1, HD, 2)
    in_maps = []
    for c in range(N_CORES):
        g0, g1 = HPC * c, HPC * c + 1
        # w_qk columns: [qh0 ev | qh1 ev | kh0 ev | kh1 ev |
        #                qh0 od | qh1 od | kh0 od | kh1 od], 64 each
        w_qk = np.concatenate([
            wq[:, g0 * HD + ev], wq[:, g1 * HD + ev],
            wk[:, g0 * HD + ev], wk[:, g1 * HD + ev],
            wq[:, g0 * HD + od], wq[:, g1 * HD + od],
            wk[:, g0 * HD + od], wk[:, g1 * HD + od],
        ], axis=1).astype(NPBF16)
        cs = slice(c * CW, (c + 1) * CW)
        w_v = wv[:, cs].astype(NPBF16)
        # rows of wo for this core's heads (2c, 2c+1); v/o are not roped,
        # so natural head-dim order
        wo_c = wo[cs, :].astype(NPBF16)
        in_maps.append({
            "x_q0": x_q0,
            "x_q1": x_q1,
            "x_h2": x_h2,
            "w_qk": _kpack(w_qk, NKC, 2 * CW),
            "w_v": _kpack(w_v, NKC, CW),
            "wo_c": _kpack(wo_c, HPC, DM),
            "cos_b": cos_b,
            "sin_b": sin_b,
            "mask": mask,
        })
    return in_maps


def assemble_output(results):
    acc = np.zeros((DM, S), np.float32)
    for r in results:
        acc += np.asarray(r["outT"], np.float32)
    return np.ascontiguousarray(acc.T).reshape(1, S, DM)


def kernel(x, wq, wk, wv, wo, freqs_cos, freqs_sin):
    nc = _get_nc()
    in_maps = make_in_maps(x, wq, wk, wv, wo, freqs_cos, freqs_sin)
    res = run_bass_kernel_spmd(nc, in_maps, core_ids=list(range(N_CORES)))
    return assemble_output(res.results)


if __name__ == "__main__":
    rng = np.random.default_rng(0)
    ins = {
        "x": rng.standard_normal((1, S, DM)).astype(np.float32),
        "wq": (rng.standard_normal((DM, DM)) / np.sqrt(DM)).astype(np.float32),
        "wk": (rng.standard_normal((DM, DM)) / np.sqrt(DM)).astype(np.float32),
        "wv": (rng.standard_normal((DM, DM)) / np.sqrt(DM)).astype(np.float32),
        "wo": (rng.standard_normal((DM, DM)) / np.sqrt(DM)).astype(np.float32),
        "freqs_cos": rng.random((S, HD // 2)).astype(np.float32),
        "freqs_sin": rng.random((S, HD // 2)).astype(np.float32),
    }
    out = kernel(**ins)
    print("out", out.shape, out.dtype, np.abs(out).mean())


# revision 69
# speedup vs baseline: 1.0567x; 1.0341x over previous
"""Trainium2 Bass kernel for Llama-like attention (16 heads, tanh softcap, RoPE).

Sharding: tensor-parallel over heads, fully COLLECTIVE-FREE. Each of the 8
cores computes 2 heads end to end; the only cross-core combine (the sum over
heads after wo) happens on the host, so no core ever waits on another and the
device time is pure single-core compute.

Per core:
  - all inputs are host-packed to [128, ...] k-chunk-major layouts so each
    tensor is ONE contiguous full-rate DMA, issued in first-use order.
  - q/k/v projections in natural [s, d] layout, f32 PSUM. wq/wk columns are
    host-interleaved as [qh0|qh1|kh0|kh1 evens, same odds], so RoPE is 6
    contiguous [128, 256] DVE ops per s-chunk reading PSUM directly (the
    head-dim permutation to [evens|odds] cancels inside q.k).
  - attention with scores transposed ([kj, qi]) so softmaxed probabilities
    feed the PV matmul as the moving operand. tanh softcap bounds scores,
    so softmax needs no row-max pass: p = exp(50*tanh(qk/(50*sqrt(hd)))),
    l = ones-row matmul, o = p@v / l. Fully-masked diagonal spans skip the
    activations; a [zeros|triangle] mask fuses zeroing + causal masking
    into one multiply.
  - output projection is a PARTIAL over the full output width:
    outT_c = wo[local head rows, :]^T @ oT_local ([DM, S] fp16); the host
    sums the 8 partials in f32.
"""

import os
import sys

for _p in ("/root/.axon_site/_ro/trn_rl_repo", "/opt/trn_rl_repo"):
    if os.path.isdir(_p) and _p not in sys.path:
        sys.path.append(_p)

import numpy as np
import ml_dtypes
from contextlib import ExitStack

import concourse.bass as bass
import concourse.bacc as bacc
import concourse.mybir as mybir
import concourse.tile as tile
from concourse.bass_utils import run_bass_kernel_spmd
from concourse.masks import make_identity

BF16 = mybir.dt.bfloat16
F16 = mybir.dt.float16
F32 = mybir.dt.float32
NPBF16 = ml_dtypes.bfloat16

N_CORES = 8
S = 2048          # sequence length
DM = 2048         # model dim
H = 16            # heads
HD = 128          # head dim
HPC = H // N_CORES  # heads per core = 2
CW = HPC * HD     # per-core projection width = 256
P = 128
QT = 512          # query tile (free dim of attention matmuls)
NQT = S // QT     # 4 query tiles per head
NSC = S // P      # 16 sequence chunks
NKC = DM // P     # 16 contraction chunks
SOFTCAP = 50.0
C1 = 1.0 / (SOFTCAP * np.sqrt(HD))

Tanh = mybir.ActivationFunctionType.Tanh
Exp = mybir.ActivationFunctionType.Exp


def build_nc(reps=1, single=False):
    nc = bacc.Bacc("TRN2", target_bir_lowering=False, num_devices=N_CORES)

    # all inputs host-packed to [P, ...] so each is one contiguous DMA
    xq0_d = nc.dram_tensor("x_q0", [P, NKC * 512], BF16, kind="ExternalInput")
    xq1_d = nc.dram_tensor("x_q1", [P, NKC * 512], BF16, kind="ExternalInput")
    xh2_d = nc.dram_tensor("x_h2", [P, NKC * 1024], BF16, kind="ExternalInput")
    wqk_d = nc.dram_tensor("w_qk", [P, NKC * 2 * CW], BF16, kind="ExternalInput")
    wv_d = nc.dram_tensor("w_v", [P, NKC * CW], BF16, kind="ExternalInput")
    wo_d = nc.dram_tensor("wo_c", [P, HPC * DM], BF16, kind="ExternalInput")
    # cos/sin pre-packed on host to [P, NSC*4*HD/2] so one contiguous DMA each
    cos_d = nc.dram_tensor("cos_b", [P, NSC * 4 * (HD // 2)], BF16,
                           kind="ExternalInput")
    sin_d = nc.dram_tensor("sin_b", [P, NSC * 4 * (HD // 2)], BF16,
                           kind="ExternalInput")
    mask_d = nc.dram_tensor("mask", [P, 4 * P], BF16, kind="ExternalInput")
    out_d = nc.dram_tensor("outT", [DM, S], F16, kind="ExternalOutput")

    with tile.TileContext(nc) as tc:
        for _rep in range(reps):
            _emit_body(nc, tc, xq0_d, xq1_d, xh2_d, wqk_d, wv_d, wo_d,
                       cos_d, sin_d, mask_d, out_d)
    nc.compile()
    return nc


def _emit_body(nc, tc, xq0_d, xq1_d, xh2_d, wqk_d, wv_d, wo_d, cos_d,
               sin_d, mask_d, out_d):
        with ExitStack() as ctx:
            # ---------- persistent SBUF ----------
            persist = ctx.enter_context(tc.tile_pool(name="persist", bufs=1))
            qT = [persist.tile([P, S], BF16, name=f"qT{h}") for h in range(HPC)]
            kT = [persist.tile([P, S], BF16, name=f"kT{h}") for h in range(HPC)]
            v_sb = [persist.tile([P, S], BF16, name=f"v{h}") for h in range(HPC)]
            oT = [persist.tile([P, S], BF16, name=f"oT{h}") for h in range(HPC)]
            mask_sb = persist.tile([P, 4 * P], BF16, name="mask")
            ident = persist.tile([P, P], BF16, name="ident")
            ones_bf = persist.tile([P, 1], BF16, name="ones")
            # cos/sin duplicated 4x on host: one [P, 256] slice ropes
            # q-h0|q-h1|k-h0|k-h1 in a single op
            cos_sb = persist.tile([P, NSC, 4 * (HD // 2)], BF16, name="cos")
            sin_sb = persist.tile([P, NSC, 4 * (HD // 2)], BF16, name="sin")
            wo_sb = persist.tile([P, HPC, DM], BF16, name="wo")

            make_identity(nc, ident[:])
            nc.vector.memset(ones_bf[:], 1.0)

            # shared attention SBUF pools (used both interleaved with phase A
            # and in the tail); pT bufs zeroed once up front: diagonal
            # mask-muls zero stale spans, safe only for finite stale data
            thp = ctx.enter_context(tc.tile_pool(name="tanh", bufs=4))
            pp = ctx.enter_context(tc.tile_pool(name="pT", bufs=9))
            np_ = ctx.enter_context(tc.tile_pool(name="norm", bufs=2))
            outp = ctx.enter_context(tc.tile_pool(name="out", bufs=3))

            def emit_c_half(m, nbase, acc_pool, act_copy, st_pool=None):
                """one [P, 1024] block of the partial output projection: two
                512-wide PSUM accs staged into one fp16 tile, one DMA. Copies
                go to DVE when ACT is the busy engine (attention tail)."""
                osb = (st_pool or outp).tile([P, 2 * QT], F16, name="osb")
                for i in range(2):
                    n = nbase + i
                    acc = acc_pool.tile([P, QT], F32, name="acc")
                    for j in range(HPC):
                        nc.tensor.matmul(
                            acc[:], wo_sb[:, j, m * P:(m + 1) * P],
                            oT[j][:, n * QT:(n + 1) * QT],
                            start=(j == 0), stop=(j == HPC - 1),
                        )
                    if act_copy and i == 0:
                        nc.scalar.copy(osb[:, 0:QT], acc[:])
                    else:
                        nc.vector.tensor_copy(
                            osb[:, i * QT:(i + 1) * QT], acc[:])
                nc.sync.dma_start(
                    out=out_d[m * P:(m + 1) * P,
                              nbase * QT:(nbase + 2) * QT],
                    in_=osb[:])
            # ---------- phase A: qkv projections + rope + transpose ----------
            with ExitStack() as ctxA:
                xp = ctxA.enter_context(tc.tile_pool(name="xT", bufs=1))
                wp = ctxA.enter_context(tc.tile_pool(name="w", bufs=1))
                rp = ctxA.enter_context(tc.tile_pool(name="rope", bufs=3))
                tmp = ctxA.enter_context(tc.tile_pool(name="ropetmp", bufs=4))
                qk_ps = ctxA.enter_context(
                    tc.tile_pool(name="qk_ps", bufs=1, space="PSUM")
                )
                v_ps = ctxA.enter_context(
                    tc.tile_pool(name="v_ps", bufs=1, space="PSUM")
                )
                tp_ps = ctxA.enter_context(
                    tc.tile_pool(name="tp_ps", bufs=2, space="PSUM")
                )

                # every input is one contiguous full-rate DMA, ordered by
                # first use: w_qk+x_q0 gate s-chunk 0, then w_v, cos/sin
                # (first rope), the rest of x, mask/wo (phase B/C)
                xq0_sb = xp.tile([P, NKC, 512], BF16, name="xq0")
                xq1_sb = xp.tile([P, NKC, 512], BF16, name="xq1")
                xh2_sb = xp.tile([P, NKC, 1024], BF16, name="xh2")
                wt_sb = wp.tile([P, NKC, 2 * CW], BF16, name="wqk")
                wv_sb2 = wp.tile([P, NKC, CW], BF16, name="wv")
                QK = NKC // 4
                for g in range(4):
                    k0, k1 = g * QK, (g + 1) * QK
                    nc.sync.dma_start(out=wt_sb[:, k0:k1, :],
                                      in_=wqk_d[:, k0 * 2 * CW:k1 * 2 * CW])
                    nc.sync.dma_start(out=xq0_sb[:, k0:k1, :],
                                      in_=xq0_d[:, k0 * 512:k1 * 512])
                nc.sync.dma_start(out=wv_sb2[:], in_=wv_d[:])
                nc.sync.dma_start(out=cos_sb[:], in_=cos_d[:])
                nc.sync.dma_start(out=sin_sb[:], in_=sin_d[:])
                nc.sync.dma_start(out=xq1_sb[:], in_=xq1_d[:])
                nc.sync.dma_start(out=xh2_sb[:], in_=xh2_d[:])
                nc.sync.dma_start(out=mask_sb[:], in_=mask_d[:])
                nc.sync.dma_start(out=wo_sb[:], in_=wo_d[:])

                HW = HD // 2  # 64

                def emit_proj_chunk(sc):
                    ps = qk_ps.tile([P, 2 * CW], F32, name="qk")
                    psv = v_ps.tile([P, CW], F32, name="v")
                    xsb = xq0_sb if sc < 4 else (xq1_sb if sc < 8 else xh2_sb)
                    xo = (sc % 4) * P if sc < 8 else (sc - 8) * P
                    for k in range(NKC):
                        lhsT = xsb[:, k, xo:xo + P]
                        nc.tensor.matmul(
                            ps[:, 0:512], lhsT, wt_sb[:, k, :],
                            start=(k == 0), stop=(k == NKC - 1),
                        )
                    for k in range(NKC):
                        lhsT = xsb[:, k, xo:xo + P]
                        nc.tensor.matmul(
                            psv[:], lhsT, wv_sb2[:, k, :],
                            start=(k == 0), stop=(k == NKC - 1),
                        )
                    for h in range(HPC):
                        nc.scalar.copy(
                            v_sb[h][:, sc * P:(sc + 1) * P],
                            psv[:, h * HD:(h + 1) * HD],
                        )
                    c_ap = cos_sb[:, sc, :]
                    s_ap = sin_sb[:, sc, :]
                    # rope reads the projection PSUM directly; the host packs
                    # w_qk columns as [qh0|qh1|kh0|kh1 evens, same odds] so
                    # x0/x1 are contiguous [P, 256] and one mul covers q+k of
                    # both heads
                    qkst = rp.tile([P, 512], BF16, name="qkst")
                    nc.vector.tensor_copy(qkst[:], ps[:, 0:512])
                    x0 = qkst[:, 0:256]
                    x1 = qkst[:, 256:512]
                    rot = rp.tile([P, 4, 2, HW], BF16, name="rot")
                    t1 = tmp.tile([P, 256], BF16, name="t1")
                    t2 = tmp.tile([P, 256], BF16, name="t2")
                    nc.vector.tensor_mul(t1[:], x0, c_ap)
                    nc.vector.tensor_mul(t2[:], x1, s_ap)
                    nc.vector.tensor_sub(rot[:, :, 0, :], t1[:], t2[:])
                    t3 = tmp.tile([P, 256], BF16, name="t3")
                    t4 = tmp.tile([P, 256], BF16, name="t4")
                    nc.vector.tensor_mul(t3[:], x0, s_ap)
                    nc.vector.tensor_mul(t4[:], x1, c_ap)
                    nc.vector.tensor_add(rot[:, :, 1, :], t3[:], t4[:])
                    # head-dim order becomes [evens, odds] for both q and k,
                    # which cancels in q.k
                    for srcg, rotT in ((0, qT), (1, kT)):
                        for h in range(HPC):
                            g = srcg * 2 + h
                            tp = tp_ps.tile([P, P], BF16, name="tp")
                            nc.tensor.transpose(
                                tp[:], rot[:, g, :, :], ident[:])
                            nc.scalar.copy(
                                rotT[h][:, sc * P:(sc + 1) * P], tp[:])

                def softcap(pT, sp, lo, hi, th_pool):
                    """pT[:, lo:hi] = exp(50*tanh(sp*C1)), both on ACT."""
                    th = th_pool.tile([P, 2 * QT], F32, name="th")
                    nc.scalar.activation(th[:, lo:hi], sp[:, lo:hi], Tanh,
                                         scale=C1)
                    nc.scalar.activation(pT[:, lo:hi], th[:, lo:hi], Exp,
                                         scale=SOFTCAP)

                def sc_block(h, t, s_pool, th_pool, p_pool):
                    """scores + softcapped exp for every causal chunk-pair of
                    query tile t; returns the probability tiles for pv_block"""
                    q_ap = qT[h][:, t * QT:(t + 1) * QT]
                    plist = []
                    for p in range(2 * t + 2):
                        sp = s_pool.tile([P, 2 * QT], F32, name="sp")
                        for i in range(2):
                            kc = 2 * p + i
                            nc.tensor.matmul(
                                sp[:, i * QT:(i + 1) * QT],
                                kT[h][:, kc * P:(kc + 1) * P], q_ap,
                                start=True, stop=True,
                            )
                        pT = p_pool.tile([P, 2 * QT], BF16, name="pTt")
                        u0 = 2 * (p - 2 * t)
                        if u0 < 0:
                            softcap(pT, sp, 0, 2 * QT, th_pool)
                        else:
                            # diagonal pair: chunk u=u0+i is fully masked for
                            # in-tile queries < 128*u; softcap only the live
                            # span, then one mul against [zeros|triangle]
                            # zeroes the masked span and applies the triangle
                            for i in range(2):
                                zs = (u0 + i) * P
                                c0 = i * QT
                                softcap(pT, sp, c0 + zs, c0 + QT, th_pool)
                                nc.gpsimd.tensor_mul(
                                    pT[:, c0:c0 + zs + P],
                                    pT[:, c0:c0 + zs + P],
                                    mask_sb[:, 3 * P - zs:4 * P])
                        plist.append(pT)
                    return plist

                def pv_block(h, t, plist, o_pool, l_pool, n_pool):
                    """o = p@v accumulation, l = ones-row sums, then the
                    softmax normalization into oT"""
                    o_acc = o_pool.tile([P, QT], F32, name="o_acc")
                    l_acc = l_pool.tile([1, QT], F32, name="l_acc")
                    npair = len(plist)
                    for p, pT in enumerate(plist):
                        for i in range(2):
                            kc = 2 * p + i
                            last = (p == npair - 1 and i == 1)
                            nc.tensor.matmul(
                                o_acc[:],
                                v_sb[h][:, kc * P:(kc + 1) * P],
                                pT[:, i * QT:(i + 1) * QT],
                                start=(kc == 0), stop=last,
                            )
                            nc.tensor.matmul(
                                l_acc[:], ones_bf[:, 0:1],
                                pT[:, i * QT:(i + 1) * QT],
                                start=(kc == 0), stop=last,
                            )
                    recip = n_pool.tile([1, QT], F32, name="recip")
                    nc.vector.reciprocal(recip[:], l_acc[:])
                    bcast = n_pool.tile([P, QT], F32, name="bcast")
                    nc.gpsimd.partition_broadcast(bcast[:], recip[:])
                    nc.vector.tensor_mul(
                        oT[h][:, t * QT:(t + 1) * QT], o_acc[:], bcast[:])

                for sc in range(4):
                    emit_proj_chunk(sc)

                # query tiles t0..t2 interleave with the remaining projection
                # chunks: tile t needs only s-chunks 0..4t+3, and ACT chews
                # tanh/exp while PE is busy projecting
                with ExitStack() as ctxAB:
                    s1 = ctxAB.enter_context(
                        tc.tile_pool(name="s1_ps", bufs=1, space="PSUM"))
                    o1 = ctxAB.enter_context(
                        tc.tile_pool(name="o1_ps", bufs=1, space="PSUM"))
                    l1 = ctxAB.enter_context(
                        tc.tile_pool(name="l1_ps", bufs=1, space="PSUM"))
                    th1, pp1, np1 = thp, pp, np_

                    emit_proj_chunk(4)
                    pl = sc_block(0, 0, s1, th1, pp1)
                    emit_proj_chunk(5)
                    pv_block(0, 0, pl, o1, l1, np1)
                    pl = sc_block(1, 0, s1, th1, pp1)
                    emit_proj_chunk(6)
                    pv_block(1, 0, pl, o1, l1, np1)
                    emit_proj_chunk(7)
                    emit_proj_chunk(8)
                    pl = sc_block(0, 1, s1, th1, pp1)
                    emit_proj_chunk(9)
                    pv_block(0, 1, pl, o1, l1, np1)
                    pl = sc_block(1, 1, s1, th1, pp1)
                    emit_proj_chunk(10)
                    pv_block(1, 1, pl, o1, l1, np1)
                    emit_proj_chunk(11)
                    emit_proj_chunk(12)
                    pl = sc_block(0, 2, s1, th1, pp1)
                    emit_proj_chunk(13)
                    pv_block(0, 2, pl, o1, l1, np1)
                    pl = sc_block(1, 2, s1, th1, pp1)
                    emit_proj_chunk(14)
                    pv_block(1, 2, pl, o1, l1, np1)
                    emit_proj_chunk(15)

            # ---------- phase B tail: the last query tile per head ----------
            with ExitStack() as ctxB:
                s_ps = ctxB.enter_context(
                    tc.tile_pool(name="s_ps", bufs=2, space="PSUM"))
                o_ps = ctxB.enter_context(
                    tc.tile_pool(name="o_ps", bufs=1, space="PSUM"))
                l_ps = ctxB.enter_context(
                    tc.tile_pool(name="l_ps", bufs=1, space="PSUM"))
                woB_ps = ctxB.enter_context(
                    tc.tile_pool(name="woB_ps", bufs=2, space="PSUM"))
                # output columns for query tiles t0/t1 are final, so the
                # first output-projection half fills the tail's PE slack
                # (copies on DVE — ACT is the tail's bottleneck)
                pl = sc_block(0, 3, s_ps, thp, pp)
                for m in range(0, NKC // 2):
                    emit_c_half(m, 0, woB_ps, act_copy=False)
                pv_block(0, 3, pl, o_ps, l_ps, np_)
                pl = sc_block(1, 3, s_ps, thp, pp)
                for m in range(NKC // 2, NKC):
                    emit_c_half(m, 0, woB_ps, act_copy=False)
                pv_block(1, 3, pl, o_ps, l_ps, np_)

            # ---------- phase C: remaining output projection half ----------
            with ExitStack() as ctxC:
                wo_ps = ctxC.enter_context(
                    tc.tile_pool(name="wo_ps", bufs=6, space="PSUM"))
                outp2 = ctxC.enter_context(tc.tile_pool(name="out2", bufs=6))
                for m in range(NKC):
                    emit_c_half(m, 2, wo_ps, act_copy=True, st_pool=outp2)


_NC_CACHE = None


def _get_nc():
    global _NC_CACHE
    if _NC_CACHE is None:
        _NC_CACHE = build_nc()
    return _NC_CACHE


def _kpack(a, kc, kw):
    """[kc*128, kc_width] row-major -> [128, kc*kc_width] k-chunk-packed"""
    a = np.ascontiguousarray(a)
    return np.ascontiguousarray(
        a.reshape(kc, P, kw).transpose(1, 0, 2).reshape(P, kc * kw))


def make_in_maps(x, wq, wk, wv, wo, freqs_cos, freqs_sin):
    x = np.asarray(x, np.float32).reshape(S, DM)
    wq = np.asarray(wq, np.float32)
    wk = np.asarray(wk, np.float32)
    wv = np.asarray(wv, np.float32)
    wo = np.asarray(wo, np.float32)
    xT = np.ascontiguousarray(x.T).astype(NPBF16)  # [DM, S]
    x_q0 = _kpack(xT[:, 0:512], NKC, 512)
    x_q1 = _kpack(xT[:, 512:1024], NKC, 512)
    x_h2 = _kpack(xT[:, 1024:S], NKC, 1024)

    # cos/sin packed to [P, NSC*4*64]: dest[p, (sc*4+g)*64+f] = src[sc*128+p, f]
    # (duplicated over g=0..3 so one [P,256] slice ropes qh0|qh1|kh0|kh1)
    def _pack(t):
        t = np.asarray(t, np.float32).reshape(NSC, P, HD // 2)
        t4 = np.repeat(t[:, :, None, :], 4, axis=2)  # [NSC, P, 4, 64]
        return np.ascontiguousarray(
            t4.transpose(1, 0, 2, 3).reshape(P, NSC * 4 * (HD // 2))
        ).astype(NPBF16)

    cos_b = _pack(freqs_cos)
    sin_b = _pack(freqs_sin)
    # [P, 4P] = [zeros(3P) | triangle(P)]: slicing the tail [3P-zs : 4P]
    # gives zs zeros then the causal triangle (keep key-off i <= query-off j)
    i_idx = np.arange(P)[:, None]
    j_idx = np.arange(4 * P)[None, :]
    mask = (np.logical_and(j_idx >= 3 * P, i_idx <= j_idx - 3 * P)
            ).astype(NPBF16)
    ev = np.arange(0, HD, 2)
    od = np.arange(1, HD, 2)
    in_maps = []
    for c in range(N_CORES):
        g0, g1 = HPC * c, HPC * c + 1
        # w_qk columns: [qh0 ev | qh1 ev | kh0 ev | kh1 ev |
        #                qh0 od | qh1 od | kh0 od | kh1 od], 64 each
        w_qk = np.concatenate([
            wq[:, g0 * HD + ev], wq[:, g1 * HD + ev],
            wk[:, g0 * HD + ev], wk[:, g1 * HD + ev],
            wq[:, g0 * HD + od], wq[:, g1 * HD + od],
            wk[:, g0 * HD + od], wk[:, g1 * HD + od],
        ], axis=1).astype(NPBF16)
        cs = slice(c * CW, (c + 1) * CW)
        w_v = wv[:, cs].astype(NPBF16)
        # rows of wo for this core's heads (2c, 2c+1); v/o are not roped,
        # so natural head-dim order
        wo_c = wo[cs, :].astype(NPBF16)
        in_maps.append({
            "x_q0": x_q0,
            "x_q1": x_q1,
            "x_h2": x_h2,
            "w_qk": _kpack(w_qk, NKC, 2 * CW),
            "w_v": _kpack(w_v, NKC, CW),
            "wo_c": _kpack(wo_c, HPC, DM),
            "cos_b": cos_b,
            "sin_b": sin_b,
            "mask": mask,
        })
    return in_maps


def assemble_output(results):
    acc = np.zeros((DM, S), np.float32)
    for r in results:
        acc += np.asarray(r["outT"], np.float32)
    return np.ascontiguousarray(acc.T).reshape(1, S, DM)


def kernel(x, wq, wk, wv, wo, freqs_cos, freqs_sin):
    nc = _get_nc()
    in_maps = make_in_maps(x, wq, wk, wv, wo, freqs_cos, freqs_sin)
    res = run_bass_kernel_spmd(nc, in_maps, core_ids=list(range(N_CORES)))
    return assemble_output(res.results)


if __name__ == "__main__":
    rng = np.random.default_rng(0)
    ins = {
        "x": rng.standard_normal((1, S, DM)).astype(np.float32),
        "wq": (rng.standard_normal((DM, DM)) / np.sqrt(DM)).astype(np.float32),
        "wk": (rng.standard_normal((DM, DM)) / np.sqrt(DM)).astype(np.float32),
        "wv": (rng.standard_normal((DM, DM)) / np.sqrt(DM)).astype(np.float32),
        "wo": (rng.standard_normal((DM, DM)) / np.sqrt(DM)).astype(np.float32),
        "freqs_cos": rng.random((S, HD // 2)).astype(np.float32),
        "freqs_sin": rng.random((S, HD // 2)).astype(np.float32),
    }
    out = kernel(**ins)
    print("out", out.shape, out.dtype, np.abs(out).mean())
